# revision 1
# baseline (speedup 1.0000x reference)
"""Bass/Trainium2 kernel for ContextHypergraphAttention.

Math: the reference computes softmax(Q K^T / sqrt(E) + bias) @ V where the
context bias is constant along the softmax axis, so softmax is invariant to
it and the context path is dropped entirely.  Per (batch, query-half) shard
(8 cores = 4 batches x 2 query halves) each core runs a single-head
attention over its 2048 query rows against the full 4096 keys of its batch:

  prologue: KT = Wk^T-proj of X^T (+bk), QT likewise (scaled 1/sqrt(E)),
            V tiles [m,128f]
  loop over 16 q-tiles: S = QT_tile^T @ KT (PSUM, f32) -> ACT exp with
            per-partition accum (rowsum) -> DVE normalize P by 1/rowsum ->
            batched SBUF->SBUF xbar DMA transpose of P -> per 4-qtile group:
            AV matmul accumulating out^T[f, q] over 32 key tiles
  epilogue: out^T + bv -> DRAM; host transposes back.

All matmuls bf16 (f32 PSUM).  Softmax skips the max-subtraction: logits are
~N(0, 0.33^2) so exp never overflows; softmax(x) == softmax(x - max) exactly.
"""

import numpy as np
import ml_dtypes
from contextlib import ExitStack

import concourse.bass as bass
import concourse.tile as tile
from concourse import bacc, mybir
from concourse.bass_utils import run_bass_kernel_spmd

B, N, E = 4, 4096, 128
NQ = N // 2          # queries per core
N_CORES = 8
MT = N // 128        # 32 key tiles
QT_TILES = NQ // 128  # 16 q tiles
QG = 4               # q-tiles per AV group
BF16 = ml_dtypes.bfloat16

_CACHE = {}


def _emit(tc):
    nc = tc.nc
    f32 = mybir.dt.float32
    bf16 = mybir.dt.bfloat16
    Exp = mybir.ActivationFunctionType.Exp
    X = mybir.AxisListType.X

    ap = {n: nc.in_aps[n] for n in nc.in_aps}

    with ExitStack() as ctx:
        consts = ctx.enter_context(tc.tile_pool(name="consts", bufs=1))

        wq_sb = consts.tile([E, E], bf16)
        nc.sync.dma_start(wq_sb[:], ap["wq"])
        wk_sb = consts.tile([E, E], bf16)
        nc.sync.dma_start(wk_sb[:], ap["wk"])
        wv_sb = consts.tile([E, E], bf16)
        nc.sync.dma_start(wv_sb[:], ap["wv"])
        bq_sb = consts.tile([E, 1], f32)
        nc.sync.dma_start(bq_sb[:], ap["bq"])
        bk_sb = consts.tile([E, 1], f32)
        nc.sync.dma_start(bk_sb[:], ap["bk"])
        bv_sb = consts.tile([E, 1], f32)
        nc.sync.dma_start(bv_sb[:], ap["bv"])
        xt_sb = consts.tile([E, N], bf16)
        nc.sync.dma_start(xt_sb[:], ap["xt"])
        xq_sb = consts.tile([E, NQ], bf16)
        nc.sync.dma_start(xq_sb[:], ap["xq"])

        kt_sb = consts.tile([E, N], bf16)
        qt_sb = consts.tile([E, NQ], bf16)
        v_sb = consts.tile([128, MT, E], bf16)
        ot_ap = nc.out_aps["ot"]

        # ---- projections ----
        with tc.tile_pool(name="proj_psum", bufs=2, space="PSUM") as pp:
            for j in range(N // 512):
                ps = pp.tile([128, 512], f32, tag="kq", name=f"pk{j}")
                nc.tensor.matmul(ps[:], wk_sb[:], xt_sb[:, j * 512:(j + 1) * 512],
                                 start=True, stop=True)
                nc.vector.tensor_scalar_add(
                    kt_sb[:, j * 512:(j + 1) * 512], ps[:], bk_sb[:])
            for j in range(NQ // 512):
                ps = pp.tile([128, 512], f32, tag="kq", name=f"pq{j}")
                nc.tensor.matmul(ps[:], wq_sb[:], xq_sb[:, j * 512:(j + 1) * 512],
                                 start=True, stop=True)
                nc.vector.tensor_scalar_add(
                    qt_sb[:, j * 512:(j + 1) * 512], ps[:], bq_sb[:])
            for t in range(MT):
                ps = pp.tile([128, E], f32, tag="v", name=f"pv{t}")
                nc.tensor.matmul(ps[:], xt_sb[:, t * 128:(t + 1) * 128], wv_sb[:],
                                 start=True, stop=True)
                nc.vector.tensor_copy(v_sb[:, t, :], ps[:])

        # ---- main attention loop ----
        CHUNKS = [(0, 1536), (1536, 1536), (3072, 1024)]
        spool = ctx.enter_context(tc.tile_pool(name="s_psum", bufs=2, space="PSUM"))
        avpool = ctx.enter_context(tc.tile_pool(name="av_psum", bufs=2, space="PSUM"))
        ppool = ctx.enter_context(tc.tile_pool(name="p", bufs=2))
        pnpool = ctx.enter_context(tc.tile_pool(name="pn", bufs=2))
        ptpool = ctx.enter_context(tc.tile_pool(name="pt", bufs=2))
        rpool = ctx.enter_context(tc.tile_pool(name="rs", bufs=3))
        opool = ctx.enter_context(tc.tile_pool(name="o", bufs=2))

        for g in range(QT_TILES // QG):
            pt_sb = ptpool.tile([128, MT, QG * 128], bf16, tag="pt", name=f"pt{g}")
            for li in range(QG):
                i = g * QG + li
                qti = qt_sb[:, i * 128:(i + 1) * 128]
                p_sb = ppool.tile([128, N], bf16, tag="p", name=f"p{i}")
                rs_parts = rpool.tile([128, len(CHUNKS)], f32, tag="rsp",
                                      name=f"rsp{i}")
                for c, (off, csz) in enumerate(CHUNKS):
                    s_ps = spool.tile([128, 1536], f32, tag="s", name=f"s{i}_{c}")
                    for so in range(0, csz, 512):
                        nc.tensor.matmul(
                            s_ps[:, so:so + 512], qti,
                            kt_sb[:, off + so:off + so + 512],
                            start=True, stop=True)
                    nc.scalar.activation(
                        p_sb[:, off:off + csz], s_ps[:, :csz], Exp,
                        accum_out=rs_parts[:, c:c + 1])
                rs = rpool.tile([128, 1], f32, tag="rs", name=f"rs{i}")
                nc.vector.reduce_sum(rs[:], rs_parts[:], axis=X)
                rcp = rpool.tile([128, 1], f32, tag="rcp", name=f"rcp{i}")
                nc.vector.reciprocal(rcp[:], rs[:])
                pn_sb = pnpool.tile([128, N], bf16, tag="pn", name=f"pn{i}")
                nc.vector.tensor_scalar_mul(pn_sb[:], p_sb[:], rcp[:])
                # batched xbar transpose: out[p, t, q] = pn[q, t*128 + p]
                nc.sync.dma_start_transpose(
                    pt_sb[:, :, li * 128:(li + 1) * 128], pn_sb[:])

            av = avpool.tile([128, QG * 128], f32, tag="av", name=f"av{g}")
            for t in range(MT):
                nc.tensor.matmul(av[:], v_sb[:, t, :], pt_sb[:, t, :],
                                 start=(t == 0), stop=(t == MT - 1))
            o_sb = opool.tile([128, QG * 128], f32, tag="o", name=f"o{g}")
            nc.vector.tensor_scalar_add(o_sb[:], av[:], bv_sb[:])
            nc.sync.dma_start(ot_ap[:, g * QG * 128:(g + 1) * QG * 128], o_sb[:])


def build_nc():
    if "nc" in _CACHE:
        return _CACHE["nc"]
    nc = bacc.Bacc("TRN2", target_bir_lowering=False, debug=False,
                   num_devices=N_CORES)
    f32 = mybir.dt.float32
    bf16 = mybir.dt.bfloat16
    ins = {}
    for name, shape, dt in [
        ("xt", [E, N], bf16), ("xq", [E, NQ], bf16),
        ("wq", [E, E], bf16), ("wk", [E, E], bf16), ("wv", [E, E], bf16),
        ("bq", [E, 1], f32), ("bk", [E, 1], f32), ("bv", [E, 1], f32),
    ]:
        ins[name] = nc.dram_tensor(name, shape, dt, kind="ExternalInput").ap()
    nc.in_aps = ins
    nc.out_aps = {
        "ot": nc.dram_tensor("ot", [E, NQ], f32, kind="ExternalOutput").ap()}
    with tile.TileContext(nc) as tc:
        _emit(tc)
    nc.compile()
    _CACHE["nc"] = nc
    return nc


def make_in_maps(X, Wq, bq, Wk, bk, Wv, bv):
    """Per-core input dicts. Core c: batch c//2, query half c%2."""
    s = 1.0 / np.sqrt(E)
    wq_h = (Wq.astype(np.float64).T * s).astype(BF16)
    wk_h = Wk.T.astype(BF16)
    wv_h = Wv.T.astype(BF16)
    bq_h = (bq.astype(np.float64) * s).astype(np.float32).reshape(E, 1)
    bk_h = bk.astype(np.float32).reshape(E, 1)
    bv_h = bv.astype(np.float32).reshape(E, 1)
    in_maps = []
    for c in range(N_CORES):
        b, h = c // 2, c % 2
        xt = np.ascontiguousarray(X[b].T).astype(BF16)
        in_maps.append({
            "xt": xt,
            "xq": np.ascontiguousarray(xt[:, h * NQ:(h + 1) * NQ]),
            "wq": wq_h, "wk": wk_h, "wv": wv_h,
            "bq": bq_h, "bk": bk_h, "bv": bv_h,
        })
    return in_maps


def kernel(X, context, Wq, bq, Wk, bk, Wv, bv, Wc, bc):
    X = np.asarray(X, np.float32)
    nc = build_nc()
    in_maps = make_in_maps(np.asarray(X, np.float32),
                           np.asarray(Wq, np.float32), np.asarray(bq, np.float32),
                           np.asarray(Wk, np.float32), np.asarray(bk, np.float32),
                           np.asarray(Wv, np.float32), np.asarray(bv, np.float32))
    res = run_bass_kernel_spmd(nc, in_maps, core_ids=list(range(N_CORES)))
    out = np.empty((B, N, E), np.float32)
    for c in range(N_CORES):
        b, h = c // 2, c % 2
        out[b, h * NQ:(h + 1) * NQ, :] = res.results[c]["ot"].T
    return out
